# revision 28
# baseline (speedup 1.0000x reference)
"""DiffAttention kernel for 8 TRN2 NeuronCores (Bass/Tile).

Reference: x [1,128,32,32,32] stride-2 subsampled to xs [128, N=4096
tokens]; per head (4 heads, head_dim 32 split 16+16): diff_attn =
softmax(q1k1*scale) - 0.1*softmax(q2k2*scale); out = diff_attn @ v,
reshaped to [1,128,16,16,16].

Sharding: tensor-parallel over (head, query-half) = 8 cores. Each core
holds all 4096 tokens and computes attention for its 2048 queries.

Per-core dataflow (PE-paced, split-precision AV):
  - score weights folded on host: W1 = c8*wq1^T wk1, W2 likewise
    (c8 = 8*log2e*scale pre-scales scores so the fp8 Schraudolph is a
    plain add).  s_h = xs_chunk^T @ (W_h @ xq_slice): no k projection;
    both halves' score matmuls share one lhsT (the xs chunk), making
    the second LDWEIGHTS nearly free.
  - exp roles are fixed: ACT does exact exp(scale*s - 2) -> bf16 e1
    (the dominant softmax); DVE does the fp8 Schraudolph for e2
    (uint8 bits = c8*s + B, round-to-nearest, min-clamp 119 to dodge
    the e4m3 inf/NaN zone; the -2 shift keeps exp in fp8 range and
    cancels in the softmax).  One [128,512] instr per half-chunk.
  - AV: av1 = vT^T @ e1 in bf16 per chunk (precision: v and e1 in
    fp8 would double the output error); av2 via fp8 DoubleRow over
    chunk PAIRS (lhsT [128,2,128] zero-padded cols 33:128 — the ISA
    requires M=128 — rhs = two adjacent e2 ring slots [128,2,512]).
    A ones-column in vT accumulates the softmax sums.
  - psum: 6 single-bank score tiles (finest recycle granularity; a
    deeper ring keeps the PE out of tile-wait stalls) + av1 bank +
    av2 bank = 8.  av banks are single-buffered per j-block
    (start=True resets after the drain).  NOTE: never place two
    accumulation groups in one PSUM bank — it hangs the hardware.
  - keeping the PE gaplessly busy holds the fast p-state (0.42ns/col;
    any stall resets it to 0.83).  Steady state is ~1739ns per chunk
    pair: 6x216 streams + 230 DR + ~2x106 LDWEIGHTS tails.
  - output is written partition-major [p, c, d] in one contiguous DMA
    (the [n, d] layout costs ~10us of 128B descriptor generation);
    the host transposes for free.
"""

import math

import numpy as np
import ml_dtypes

import concourse.bass as bass
import concourse.mybir as mybir
import concourse.tile as tile
from concourse import bacc
from concourse.bass import ts, ds
from concourse.bass_utils import run_bass_kernel_spmd

BF16 = mybir.dt.bfloat16
I16 = mybir.dt.int16
F32 = mybir.dt.float32
FP8 = mybir.dt.float8e4
U8 = mybir.dt.uint8
NP_BF16 = ml_dtypes.bfloat16
NP_FP8 = ml_dtypes.float8_e4m3

C = 128          # channels
HEADS = 4
HD = 32          # head_dim
DH = 16          # d_half
LAMBDA = 0.1
SCALE = HD ** -0.5
R = 2
N_CORES = 8
N = 4096         # tokens after subsample
NQ = N // 2      # queries per core

MC = N // 128    # 32 key chunks of 128 tokens
ND = MC // 2     # 16 chunk pairs (double-chunks)
NJ = NQ // 512   # 4 j-blocks of 512 queries
NBS = 1024       # queries per finalize block (2 j-blocks)

# fp8e4m3 (ml_dtypes.float8_e4m3, IEEE-ish: bias 7, max 240, inf at
# bits 0x78): bits(2^y) ~= 8*(y + 7 - c), c balancing the Schraudolph
# sawtooth.  Scores are pre-scaled by C8 in the W weights so ACT
# recovers exp(scale*s) with scale ln2/8 and DVE just adds SCH8_B.
C8 = 8.0 * math.log2(math.e) * SCALE
ACT_SCALE = math.log(2.0) / 8.0
# global exponent shift (softmax-invariant): keeps exp values inside
# fp8e4m3 range (max 240; scores reach exp(6.5) otherwise)
SHIFT = 2.0
SCH8_B = 8.0 * (7.0 - 0.0579297) - 8.0 * math.log2(math.e) * SHIFT
CLAMP8 = 119.49
# bf16 Schraudolph for e1 chunks: bits = 16*ps + SCH16_B (psum is c8*s)
SCH16_A = 16.0
SCH16_B = 128.0 * (127.0 - 0.0579297) - 128.0 * math.log2(math.e) * SHIFT

# exp engine split: fraction of chunks on ACT (rest on DVE-Schraudolph)
ACT_SHARE = 0.54

ESLOTS = 8       # e8 ring slots (chunks)


def build_nc():
    """SPMD Bass program for one core = (head, query-half).

    Inputs:
      xs    [128, 4096] bf16  all tokens, channel-major
      w     [128, 288]  bf16  cols 0:128 W1^T, 128:256 W2^T, 256:288 w_v^T
      ident [128, 33]   f32   identity blocks at partitions 0:33, 64:97
    Output:
      out   [2048, 32]  f32   attention output (n, d) for this core's
                              queries
    """
    Exp = mybir.ActivationFunctionType.Exp
    DR = mybir.MatmulPerfMode.DoubleRow

    nc = bacc.Bacc()
    xs_d = nc.declare_dram_parameter("xs", [C, N], BF16, isOutput=False)
    xq_d = nc.declare_dram_parameter("xq", [C, NQ], BF16, isOutput=False)
    w_d = nc.declare_dram_parameter("w", [C, 288], BF16, isOutput=False)
    id_d = nc.declare_dram_parameter("ident", [C, 33], F32, isOutput=False)
    # partition-major output [p, c, d]: row n = c*128 + p maps to
    # out[p, c*HD : c*HD+HD]; host transposes back (free)
    out_d = nc.declare_dram_parameter("out", [C, (NQ // C) * HD], F32,
                                      isOutput=True)

    W1 = slice(0, 128)
    W2 = slice(128, 256)
    WV = slice(256, 288)

    with tile.TileContext(nc) as tc:
        with tc.tile_pool(name="mains", bufs=1) as mains:
            w_sb = mains.tile([C, 288], BF16)
            id_sb = mains.tile([C, 33], F32)
            xs_sb = mains.tile([C, N], BF16)
            xq_sb = mains.tile([C, NQ], BF16)

            # static tensors
            qq_sb = mains.tile([C, NJ * 2 * 512], BF16)   # t1|t2 per j
            vta1_sb = mains.tile([C, MC * 33], BF16)      # av1 weights v|1
            vta8_sb = mains.tile([C, ND * 2 * 128], FP8)  # av2 DR weights
            e1_sb = mains.tile([C, ESLOTS * 512], BF16)   # e1 ring
            e2_sb = mains.tile([C, ESLOTS * 512], FP8)    # e2 ring
            avs_sb = mains.tile([C, NJ * 512], F32)       # av1 p0:33, av2 p64:97
            out_sb = mains.tile([C, (NQ // 128) * HD], F32)

            vta8_v = vta8_sb[:, :].rearrange("p (d s m) -> p d s m",
                                             d=ND, s=2)
            qq_v = qq_sb[:, :].rearrange("p (j s n) -> p j s n",
                                         j=NJ, s=2)

            # activation bias AP (-SHIFT) for the exact-exp path
            bias_sb = mains.tile([C, 1], F32)
            nc.vector.memset(bias_sb[:, :], -SHIFT)
            # ones columns + zero DR pad cols, split across engines
            nc.vector.memset(vta1_sb[:, :], 1.0)
            nc.vector.memset(vta8_sb[:, 0:2048], 0.0)
            nc.gpsimd.memset(vta8_sb[:, 2048:4096], 0.0)
            nc.gpsimd.memset(vta8_v[:, :, :, 32:33], 1.0)

            # input DMAs spread across queues; w + first slabs first,
            # ident (finalize-only) last
            nc.sync.dma_start(out=w_sb[:, :], in_=w_d[:, :])
            nc.sync.dma_start(out=xq_sb[:, 0:512], in_=xq_d[:, 0:512])
            qengs = [nc.sync, nc.gpsimd, nc.scalar]
            for i in range(8):
                qengs[i % 3].dma_start(out=xs_sb[:, ts(i, 512)],
                                       in_=xs_d[:, ts(i, 512)])
            for i, off in enumerate((512, 1024, 1536)):
                qengs[i % 3].dma_start(out=xq_sb[:, ds(off, 512)],
                                       in_=xq_d[:, ds(off, 512)])
            nc.sync.dma_start(out=id_sb[:, :], in_=id_d[:, :])

            with (
                tc.tile_pool(name="sc_ps", bufs=6, space="PSUM") as spool,
                tc.tile_pool(name="a_ps", bufs=1, space="PSUM") as apool,
                tc.tile_pool(name="b_ps", bufs=1, space="PSUM") as bpool,
                tc.tile_pool(name="fin_sb", bufs=2) as fsb,
            ):

                def project_v_pair(pair):
                    # v^T for one chunk pair (256 tokens) -> vta (fp8)
                    pv = spool.tile([C, 512], F32, tag="sc", name="psv")
                    for i in range(2):
                        c = pair * 2 + i
                        nc.tensor.matmul(pv[:, ds(i * 128, HD)],
                                         lhsT=xs_sb[:, ts(c, 128)],
                                         rhs=w_sb[:, WV],
                                         start=True, stop=True)
                    srcv = pv[:, 0:256].rearrange(
                        "p (i x) -> p i x", x=128)[:, :, 0:HD]
                    dst1 = vta1_sb[:, ds(pair * 66, 66)].rearrange(
                        "p (c m) -> p c m", m=33)[:, :, 0:HD]
                    nc.scalar.copy(dst1, srcv)
                    dst8 = vta8_sb[:, ds(pair * 256, 256)].rearrange(
                        "p (c m) -> p c m", m=128)[:, :, 0:HD]
                    nc.scalar.copy(dst8, srcv)

                def project_v(slab):
                    # v^T for 4 chunks (512 tokens) -> vta (fp8)
                    pa = spool.tile([C, 512], F32, tag="sc", name="psv")
                    pb = spool.tile([C, 512], F32, tag="sc", name="psv")
                    ps_v = (pa, pb)
                    for i in range(4):
                        c = slab * 4 + i
                        nc.tensor.matmul(ps_v[i // 2][:, ds((i % 2) * 128,
                                                            HD)],
                                         lhsT=xs_sb[:, ts(c, 128)],
                                         rhs=w_sb[:, WV],
                                         start=True, stop=True)
                    for half in range(2):
                        srcv = ps_v[half][:, 0:256].rearrange(
                            "p (i x) -> p i x", x=128)[:, :, 0:HD]
                        coff = (slab * 4 + half * 2) * 33
                        dst1 = vta1_sb[:, ds(coff, 2 * 33)].rearrange(
                            "p (c m) -> p c m", m=33)[:, :, 0:HD]
                        nc.vector.tensor_copy(dst1, srcv)
                        dst8 = vta8_sb[:, ds(slab * 512 + half * 256, 256)] \
                            .rearrange("p (c m) -> p c m", m=128)[:, :, 0:HD]
                        nc.scalar.copy(dst8, srcv)

                def project_q(j, on_act):
                    # t1|t2 for j-block j -> qq (bf16)
                    pa = spool.tile([C, 512], F32, tag="sc", name="psq")
                    pb = spool.tile([C, 512], F32, tag="sc", name="psq")
                    qoff = j * 512
                    nc.tensor.matmul(pa[:, :], lhsT=w_sb[:, W1],
                                     rhs=xq_sb[:, ds(qoff, 512)],
                                     start=True, stop=True)
                    nc.tensor.matmul(pb[:, :], lhsT=w_sb[:, W2],
                                     rhs=xq_sb[:, ds(qoff, 512)],
                                     start=True, stop=True)
                    if on_act:
                        nc.scalar.copy(qq_sb[:, ds(j * 1024, 512)], pa[:, :])
                        nc.scalar.copy(qq_sb[:, ds(j * 1024 + 512, 512)],
                                       pb[:, :])
                    else:
                        nc.vector.tensor_copy(qq_sb[:, ds(j * 1024, 512)],
                                              pa[:, :])
                        nc.vector.tensor_copy(
                            qq_sb[:, ds(j * 1024 + 512, 512)], pb[:, :])

                def finalize_j(fj):
                    # avs_sb [33|33, fj*512 : +512] -> out rows
                    CQ = 4  # query chunks of 128 per j-block
                    pt1 = spool.tile([C, 512], F32, tag="sc", name="psT")
                    pt2 = spool.tile([C, 512], F32, tag="sc", name="psT")
                    psT1 = pt1[:, 0:256]
                    psT2 = pt2[:, 0:256]
                    for cq in range(CQ):
                        gq = fj * CQ + cq
                        nc.tensor.transpose(psT1[:, ds(cq * 64, 33)],
                                            avs_sb[0:33, ts(gq, 128)],
                                            id_sb[0:33, :])
                        nc.tensor.transpose(psT2[:, ds(cq * 64, 33)],
                                            avs_sb[64:97, ts(gq, 128)],
                                            id_sb[64:97, :])
                    r1_sb = fsb.tile([C, CQ], F32, tag="r1")
                    r2_sb = fsb.tile([C, CQ], F32, tag="r2")
                    sum1 = psT1.rearrange(
                        "p (c x) -> p c x", x=64)[:, :, 32:33]
                    sum2 = psT2.rearrange(
                        "p (c x) -> p c x", x=64)[:, :, 32:33]
                    nc.vector.reciprocal(r1_sb[:, :, None], sum1)
                    nc.vector.reciprocal(r2_sb[:, :, None], sum2)
                    nc.vector.tensor_scalar_mul(r2_sb[:, :], r2_sb[:, :],
                                                -LAMBDA)
                    o1_sb = fsb.tile([C, CQ * HD], F32, tag="o1")
                    o2_sb = fsb.tile([C, CQ * HD], F32, tag="o2")
                    av1t = psT1.rearrange(
                        "p (c x) -> p c x", x=64)[:, :, 0:32]
                    av2t = psT2.rearrange(
                        "p (c x) -> p c x", x=64)[:, :, 0:32]
                    o1_v = o1_sb[:, :].rearrange("p (c d) -> p c d", d=HD)
                    o2_v = o2_sb[:, :].rearrange("p (c d) -> p c d", d=HD)
                    nc.vector.tensor_tensor(
                        o1_v, av1t,
                        r1_sb[:, :, None].to_broadcast((C, CQ, HD)),
                        mybir.AluOpType.mult)
                    nc.vector.tensor_tensor(
                        o2_v, av2t,
                        r2_sb[:, :, None].to_broadcast((C, CQ, HD)),
                        mybir.AluOpType.mult)
                    nc.vector.tensor_tensor(
                        out_sb[:, ds(fj * CQ * HD, CQ * HD)],
                        o1_sb[:, :], o2_sb[:, :], mybir.AluOpType.add)
                    nc.sync.dma_start(
                        out=out_d[:, ds(fj * CQ * HD, CQ * HD)],
                        in_=out_sb[:, ds(fj * CQ * HD, CQ * HD)],
                    )

                # ---- preamble: q projection only; v-projs are emitted
                # right after j0/dc0's scores (they overlap dc0's exps)
                project_q(0, True)

                # ---- main loop
                def emit_av1(pc, first, last):
                    nc.tensor.matmul(
                        av_a[0:33, :],
                        lhsT=vta1_sb[:, ds(pc * 33, 33)],
                        rhs=e1_sb[:, ds((pc % ESLOTS) * 512, 512)],
                        start=first, stop=last,
                        skip_group_check=True)

                def emit_av2(pdc, first, last):
                    sl0 = (2 * pdc) % ESLOTS
                    rhs2 = e2_sb[:, ds(sl0 * 512, 1024)] \
                        .rearrange("p (s x) -> p s x", s=2)
                    nc.tensor.matmul(av_b[:, :],
                                     lhsT=vta8_v[:, pdc, :, :],
                                     rhs=rhs2,
                                     start=first, stop=last,
                                     perf_mode=DR,
                                     skip_group_check=True)

                av_a = av_b = None
                pending_av = None
                pending_drain = None

                for j in range(NJ):
                    for dc in range(ND):
                        c0, c1 = 2 * dc, 2 * dc + 1
                        for c in (c0, c1):
                            T1 = spool.tile([C, 512], F32, tag="sc",
                                            name="s1")
                            nc.tensor.matmul(T1[:, :],
                                             lhsT=xs_sb[:, ts(c, 128)],
                                             rhs=qq_v[:, j, 0, :],
                                             start=True, stop=True)
                            nc.scalar.activation(
                                e1_sb[:, ds((c % ESLOTS) * 512, 512)],
                                T1[:, :], Exp, bias=bias_sb[:, 0:1],
                                scale=ACT_SCALE)
                            T2 = spool.tile([C, 512], F32, tag="sc",
                                            name="s2")
                            nc.tensor.matmul(T2[:, :],
                                             lhsT=xs_sb[:, ts(c, 128)],
                                             rhs=qq_v[:, j, 1, :],
                                             start=True, stop=True)
                            nc.vector.tensor_scalar(
                                e2_sb[:, ds((c % ESLOTS) * 512, 512)]
                                .bitcast(U8), T2[:, :], SCH8_B, CLAMP8,
                                mybir.AluOpType.add, mybir.AluOpType.min)

                        if j == 0 and dc == 0:
                            for slab in range(8):
                                project_v(slab)

                        # avs of the previous double-chunk
                        if pending_av is not None:
                            pj, pdc = pending_av
                            if pdc == 0:
                                av_a = apool.tile([C, 512], F32, tag="a")
                                av_b = bpool.tile([C, 512], F32, tag="b")
                            first, last = (pdc == 0), (pdc == ND - 1)
                            emit_av1(2 * pdc, first=first, last=False)
                            emit_av1(2 * pdc + 1, first=False, last=last)
                            emit_av2(pdc, first=first, last=last)
                        pending_av = (j, dc)


                        if dc == 0 and pending_drain is not None:
                            pj = pending_drain
                            # quarter-width drains alternating engines so
                            # neither exp stream stalls for a full copy
                            for q in range(4):
                                dst_a = avs_sb[0:33,
                                               ds(pj * 512 + q * 128, 128)]
                                dst_b = avs_sb[64:97,
                                               ds(pj * 512 + q * 128, 128)]
                                src_a = av_a[0:33, ds(q * 128, 128)]
                                src_b = av_b[0:33, ds(q * 128, 128)]
                                if q % 2 == 0:
                                    nc.scalar.copy(dst_a, src_a)
                                    nc.vector.tensor_copy(dst_b, src_b)
                                else:
                                    nc.vector.tensor_copy(dst_a, src_a)
                                    nc.scalar.copy(dst_b, src_b)
                            pending_drain = None
                        if dc == 2 and j > 0:
                            finalize_j(j - 1)
                        if dc == 10 and j + 1 < NJ:
                            project_q(j + 1, True)

                    pending_drain = j

                # flush last avs + drain
                pj, pdc = pending_av
                emit_av1(2 * pdc, first=False, last=False)
                emit_av1(2 * pdc + 1, first=False, last=True)
                emit_av2(pdc, first=False, last=True)
                nc.scalar.copy(avs_sb[0:33, ts(pj, 512)], av_a[0:33, :])
                nc.vector.tensor_copy(avs_sb[64:97, ts(pj, 512)],
                                      av_b[0:33, :])

                finalize_j(NJ - 1)

    nc.compile()
    return nc


def make_identity_input():
    ident = np.zeros((C, 33), np.float32)
    ident[0:33, :] = np.eye(33, dtype=np.float32)
    ident[64:97, :] = np.eye(33, dtype=np.float32)
    return ident


def make_in_maps(x, w_qkv):
    """Host-side sharding: subsample, fold score weights, pack per core."""
    xs = np.ascontiguousarray(x[0][:, ::R, ::R, ::R]).reshape(C, N)
    xs_b = xs.astype(NP_BF16)
    ident = make_identity_input()
    in_maps = []
    for core in range(N_CORES):
        h, half = divmod(core, 2)
        blk = w_qkv[h * 96: (h + 1) * 96].astype(np.float64)
        wq, wk, wv = blk[0:32], blk[32:64], blk[64:96]
        # folded, pre-scaled score weights: t_h = (c8 wq_h^T wk_h)^T? see
        # build_nc: lhsT for the t-projection must be W_h^T = wq_h^T wk_h
        w1t = C8 * (wq[0:DH].T @ wk[0:DH])        # [128, 128]
        w2t = C8 * (wq[DH:HD].T @ wk[DH:HD])
        w = np.empty((C, 288), np.float32)
        w[:, 0:128] = w1t
        w[:, 128:256] = w2t
        w[:, 256:288] = wv.T
        in_maps.append({
            "xs": xs_b,
            "xq": np.ascontiguousarray(xs_b[:, half * NQ:(half + 1) * NQ]),
            "w": w.astype(NP_BF16),
            "ident": ident,
        })
    return in_maps


_NC_CACHE = {}


def get_nc():
    if "nc" not in _NC_CACHE:
        _NC_CACHE["nc"] = build_nc()
    return _NC_CACHE["nc"]


LAST_RESULTS = None  # BassKernelResults of the most recent kernel() call


def kernel(x, w_qkv, trace=False, **trace_kwargs):
    global LAST_RESULTS
    x = np.asarray(x)
    w_qkv = np.asarray(w_qkv)
    in_maps = make_in_maps(x, w_qkv)
    nc = get_nc()
    res = run_bass_kernel_spmd(nc, in_maps, list(range(N_CORES)),
                               trace=trace, **trace_kwargs)
    LAST_RESULTS = res
    out_hnd = np.empty((HEADS, N, HD), np.float32)
    for core in range(N_CORES):
        h, half = divmod(core, 2)
        # device wrote [p, c, d]; row n = c*128 + p
        arr = res.results[core]["out"].reshape(C, NQ // C, HD)
        out_hnd[h, half * NQ:(half + 1) * NQ, :] = \
            arr.transpose(1, 0, 2).reshape(NQ, HD)
    return out_hnd.reshape(1, C, 16, 16, 16)


# revision 29
# speedup vs baseline: 1.0004x; 1.0004x over previous
"""DiffAttention kernel for 8 TRN2 NeuronCores (Bass/Tile).

Reference: x [1,128,32,32,32] stride-2 subsampled to xs [128, N=4096
tokens]; per head (4 heads, head_dim 32 split 16+16): diff_attn =
softmax(q1k1*scale) - 0.1*softmax(q2k2*scale); out = diff_attn @ v,
reshaped to [1,128,16,16,16].

Sharding: tensor-parallel over (head, query-half) = 8 cores. Each core
holds all 4096 tokens and computes attention for its 2048 queries.

Per-core dataflow (v2 — PE-paced, fp8 AV):
  - score weights folded on host: W1 = c8*wk1^T wq1, W2 = c8*wk2^T wq2
    (c8 = 8*log2e*scale pre-scales scores so the fp8 Schraudolph is a
    plain add).  s_h = xs_chunk^T @ (W_h @ xs_qslice): no k projection,
    one shared lhsT (the xs chunk) for both halves' score matmuls.
  - exp is whole-chunk alternated between ACT (exact exp -> fp8e4m3,
    scale ln2/8) and DVE (Schraudolph: uint8 bits = c8*s + 8*(7-c),
    round-to-nearest, min-clamp 119 to avoid inf/NaN), weighted so
    both engines run just under the PE's pace.
  - AV in fp8: av1 = vT^T @ e1 per chunk (plain fp8 matmul), av2 via
    DoubleRow over chunk PAIRS (lhsT [128,2,128] zero-padded cols
    33:128, rhs = two adjacent e-slots viewed [128,2,512]) at 0.5
    cycles/col.  A ones-column in vT accumulates the softmax sums.
  - psum: 3 score tiles [128,1024] (s1|s2 per chunk) + av1 bank +
    av2 bank = 8 banks.  av banks are single-buffered per j-block
    (start=True resets); finalize transposes reuse them between
    j-blocks.
  - keeping the PE gaplessly busy (scores+av ~1385ns per chunk pair >
    exp ~1240ns) holds the fast p-state (0.42ns/col after 3us).
"""

import math

import numpy as np
import ml_dtypes

import concourse.bass as bass
import concourse.mybir as mybir
import concourse.tile as tile
from concourse import bacc
from concourse.bass import ts, ds
from concourse.bass_utils import run_bass_kernel_spmd

BF16 = mybir.dt.bfloat16
I16 = mybir.dt.int16
F32 = mybir.dt.float32
FP8 = mybir.dt.float8e4
U8 = mybir.dt.uint8
NP_BF16 = ml_dtypes.bfloat16
NP_FP8 = ml_dtypes.float8_e4m3

C = 128          # channels
HEADS = 4
HD = 32          # head_dim
DH = 16          # d_half
LAMBDA = 0.1
SCALE = HD ** -0.5
R = 2
N_CORES = 8
N = 4096         # tokens after subsample
NQ = N // 2      # queries per core

MC = N // 128    # 32 key chunks of 128 tokens
ND = MC // 2     # 16 chunk pairs (double-chunks)
NJ = NQ // 512   # 4 j-blocks of 512 queries
NBS = 1024       # queries per finalize block (2 j-blocks)

# fp8e4m3 (ml_dtypes.float8_e4m3, IEEE-ish: bias 7, max 240, inf at
# bits 0x78): bits(2^y) ~= 8*(y + 7 - c), c balancing the Schraudolph
# sawtooth.  Scores are pre-scaled by C8 in the W weights so ACT
# recovers exp(scale*s) with scale ln2/8 and DVE just adds SCH8_B.
C8 = 8.0 * math.log2(math.e) * SCALE
ACT_SCALE = math.log(2.0) / 8.0
# global exponent shift (softmax-invariant): keeps exp values inside
# fp8e4m3 range (max 240; scores reach exp(6.5) otherwise)
SHIFT = 2.0
SCH8_B = 8.0 * (7.0 - 0.0579297) - 8.0 * math.log2(math.e) * SHIFT
CLAMP8 = 119.49
# bf16 Schraudolph for e1 chunks: bits = 16*ps + SCH16_B (psum is c8*s)
SCH16_A = 16.0
SCH16_B = 128.0 * (127.0 - 0.0579297) - 128.0 * math.log2(math.e) * SHIFT

# exp engine split: fraction of chunks on ACT (rest on DVE-Schraudolph)
ACT_SHARE = 0.54

ESLOTS = 8       # e8 ring slots (chunks)


def build_nc():
    """SPMD Bass program for one core = (head, query-half).

    Inputs:
      xs    [128, 4096] bf16  all tokens, channel-major
      w     [128, 288]  bf16  cols 0:128 W1^T, 128:256 W2^T, 256:288 w_v^T
      ident [128, 33]   f32   identity blocks at partitions 0:33, 64:97
    Output:
      out   [2048, 32]  f32   attention output (n, d) for this core's
                              queries
    """
    Exp = mybir.ActivationFunctionType.Exp
    DR = mybir.MatmulPerfMode.DoubleRow

    nc = bacc.Bacc()
    xs_d = nc.declare_dram_parameter("xs", [C, N], BF16, isOutput=False)
    xq_d = nc.declare_dram_parameter("xq", [C, NQ], BF16, isOutput=False)
    w_d = nc.declare_dram_parameter("w", [C, 288], BF16, isOutput=False)
    id_d = nc.declare_dram_parameter("ident", [C, 33], F32, isOutput=False)
    # partition-major output [p, c, d]: row n = c*128 + p maps to
    # out[p, c*HD : c*HD+HD]; host transposes back (free)
    out_d = nc.declare_dram_parameter("out", [C, (NQ // C) * HD], F32,
                                      isOutput=True)

    W1 = slice(0, 128)
    W2 = slice(128, 256)
    WV = slice(256, 288)

    with tile.TileContext(nc) as tc:
        with tc.tile_pool(name="mains", bufs=1) as mains:
            w_sb = mains.tile([C, 288], BF16)
            id_sb = mains.tile([C, 33], F32)
            xs_sb = mains.tile([C, N], BF16)
            xq_sb = mains.tile([C, NQ], BF16)

            # static tensors
            qq_sb = mains.tile([C, NJ * 2 * 512], BF16)   # t1|t2 per j
            vta1_sb = mains.tile([C, MC * 33], BF16)      # av1 weights v|1
            vta8_sb = mains.tile([C, ND * 2 * 128], FP8)  # av2 DR weights
            e1_sb = mains.tile([C, ESLOTS * 512], BF16)   # e1 ring
            e2_sb = mains.tile([C, ESLOTS * 512], FP8)    # e2 ring
            avs_sb = mains.tile([C, NJ * 512], F32)       # av1 p0:33, av2 p64:97
            out_sb = mains.tile([C, (NQ // 128) * HD], F32)

            vta8_v = vta8_sb[:, :].rearrange("p (d s m) -> p d s m",
                                             d=ND, s=2)
            qq_v = qq_sb[:, :].rearrange("p (j s n) -> p j s n",
                                         j=NJ, s=2)

            # activation bias AP (-SHIFT) for the exact-exp path
            bias_sb = mains.tile([C, 1], F32)
            nc.vector.memset(bias_sb[:, :], -SHIFT)
            # ones columns + zero DR pad cols, split across engines
            nc.vector.memset(vta1_sb[:, :], 1.0)
            nc.vector.memset(vta8_sb[:, 0:2048], 0.0)
            nc.gpsimd.memset(vta8_sb[:, 2048:4096], 0.0)
            nc.gpsimd.memset(vta8_v[:, :, :, 32:33], 1.0)

            # input DMAs spread across queues; w + first slabs first,
            # ident (finalize-only) last
            nc.sync.dma_start(out=w_sb[:, :], in_=w_d[:, :])
            nc.sync.dma_start(out=xq_sb[:, 0:512], in_=xq_d[:, 0:512])
            qengs = [nc.sync, nc.gpsimd, nc.scalar]
            for i in range(8):
                qengs[i % 3].dma_start(out=xs_sb[:, ts(i, 512)],
                                       in_=xs_d[:, ts(i, 512)])
            for i, off in enumerate((512, 1024, 1536)):
                qengs[i % 3].dma_start(out=xq_sb[:, ds(off, 512)],
                                       in_=xq_d[:, ds(off, 512)])
            nc.sync.dma_start(out=id_sb[:, :], in_=id_d[:, :])

            with (
                tc.tile_pool(name="sc_ps", bufs=6, space="PSUM") as spool,
                tc.tile_pool(name="a_ps", bufs=1, space="PSUM") as apool,
                tc.tile_pool(name="b_ps", bufs=1, space="PSUM") as bpool,
                tc.tile_pool(name="fin_sb", bufs=2) as fsb,
            ):

                def project_v_pair(pair):
                    # v^T for one chunk pair (256 tokens) -> vta (fp8)
                    pv = spool.tile([C, 512], F32, tag="sc", name="psv")
                    for i in range(2):
                        c = pair * 2 + i
                        nc.tensor.matmul(pv[:, ds(i * 128, HD)],
                                         lhsT=xs_sb[:, ts(c, 128)],
                                         rhs=w_sb[:, WV],
                                         start=True, stop=True)
                    srcv = pv[:, 0:256].rearrange(
                        "p (i x) -> p i x", x=128)[:, :, 0:HD]
                    dst1 = vta1_sb[:, ds(pair * 66, 66)].rearrange(
                        "p (c m) -> p c m", m=33)[:, :, 0:HD]
                    nc.scalar.copy(dst1, srcv)
                    dst8 = vta8_sb[:, ds(pair * 256, 256)].rearrange(
                        "p (c m) -> p c m", m=128)[:, :, 0:HD]
                    nc.scalar.copy(dst8, srcv)

                def project_v(slab):
                    # v^T for 4 chunks (512 tokens) -> vta (fp8)
                    pa = spool.tile([C, 512], F32, tag="sc", name="psv")
                    pb = spool.tile([C, 512], F32, tag="sc", name="psv")
                    ps_v = (pa, pb)
                    for i in range(4):
                        c = slab * 4 + i
                        nc.tensor.matmul(ps_v[i // 2][:, ds((i % 2) * 128,
                                                            HD)],
                                         lhsT=xs_sb[:, ts(c, 128)],
                                         rhs=w_sb[:, WV],
                                         start=True, stop=True)
                    for half in range(2):
                        srcv = ps_v[half][:, 0:256].rearrange(
                            "p (i x) -> p i x", x=128)[:, :, 0:HD]
                        coff = (slab * 4 + half * 2) * 33
                        dst1 = vta1_sb[:, ds(coff, 2 * 33)].rearrange(
                            "p (c m) -> p c m", m=33)[:, :, 0:HD]
                        nc.vector.tensor_copy(dst1, srcv)
                        dst8 = vta8_sb[:, ds(slab * 512 + half * 256, 256)] \
                            .rearrange("p (c m) -> p c m", m=128)[:, :, 0:HD]
                        nc.scalar.copy(dst8, srcv)

                def project_q(j, on_act):
                    # t1|t2 for j-block j -> qq (bf16)
                    pa = spool.tile([C, 512], F32, tag="sc", name="psq")
                    pb = spool.tile([C, 512], F32, tag="sc", name="psq")
                    qoff = j * 512
                    nc.tensor.matmul(pa[:, :], lhsT=w_sb[:, W1],
                                     rhs=xq_sb[:, ds(qoff, 512)],
                                     start=True, stop=True)
                    nc.tensor.matmul(pb[:, :], lhsT=w_sb[:, W2],
                                     rhs=xq_sb[:, ds(qoff, 512)],
                                     start=True, stop=True)
                    if on_act:
                        nc.scalar.copy(qq_sb[:, ds(j * 1024, 512)], pa[:, :])
                        nc.scalar.copy(qq_sb[:, ds(j * 1024 + 512, 512)],
                                       pb[:, :])
                    else:
                        nc.vector.tensor_copy(qq_sb[:, ds(j * 1024, 512)],
                                              pa[:, :])
                        nc.vector.tensor_copy(
                            qq_sb[:, ds(j * 1024 + 512, 512)], pb[:, :])

                def finalize_j(fj):
                    # avs_sb [33|33, fj*512 : +512] -> out rows
                    CQ = 4  # query chunks of 128 per j-block
                    pt1 = spool.tile([C, 512], F32, tag="sc", name="psT")
                    pt2 = spool.tile([C, 512], F32, tag="sc", name="psT")
                    psT1 = pt1[:, 0:256]
                    psT2 = pt2[:, 0:256]
                    for cq in range(CQ):
                        gq = fj * CQ + cq
                        nc.tensor.transpose(psT1[:, ds(cq * 64, 33)],
                                            avs_sb[0:33, ts(gq, 128)],
                                            id_sb[0:33, :])
                        nc.tensor.transpose(psT2[:, ds(cq * 64, 33)],
                                            avs_sb[64:97, ts(gq, 128)],
                                            id_sb[64:97, :])
                    r1_sb = fsb.tile([C, CQ], F32, tag="r1")
                    r2_sb = fsb.tile([C, CQ], F32, tag="r2")
                    sum1 = psT1.rearrange(
                        "p (c x) -> p c x", x=64)[:, :, 32:33]
                    sum2 = psT2.rearrange(
                        "p (c x) -> p c x", x=64)[:, :, 32:33]
                    nc.vector.reciprocal(r1_sb[:, :, None], sum1)
                    nc.vector.reciprocal(r2_sb[:, :, None], sum2)
                    nc.vector.tensor_scalar_mul(r2_sb[:, :], r2_sb[:, :],
                                                -LAMBDA)
                    o1_sb = fsb.tile([C, CQ * HD], F32, tag="o1")
                    o2_sb = fsb.tile([C, CQ * HD], F32, tag="o2")
                    av1t = psT1.rearrange(
                        "p (c x) -> p c x", x=64)[:, :, 0:32]
                    av2t = psT2.rearrange(
                        "p (c x) -> p c x", x=64)[:, :, 0:32]
                    o1_v = o1_sb[:, :].rearrange("p (c d) -> p c d", d=HD)
                    o2_v = o2_sb[:, :].rearrange("p (c d) -> p c d", d=HD)
                    nc.vector.tensor_tensor(
                        o1_v, av1t,
                        r1_sb[:, :, None].to_broadcast((C, CQ, HD)),
                        mybir.AluOpType.mult)
                    nc.vector.tensor_tensor(
                        o2_v, av2t,
                        r2_sb[:, :, None].to_broadcast((C, CQ, HD)),
                        mybir.AluOpType.mult)
                    nc.vector.tensor_tensor(
                        out_sb[:, ds(fj * CQ * HD, CQ * HD)],
                        o1_sb[:, :], o2_sb[:, :], mybir.AluOpType.add)
                    nc.sync.dma_start(
                        out=out_d[:, ds(fj * CQ * HD, CQ * HD)],
                        in_=out_sb[:, ds(fj * CQ * HD, CQ * HD)],
                    )

                # ---- preamble: q projection only; v-projs are emitted
                # right after j0/dc0's scores (they overlap dc0's exps)
                project_q(0, True)

                # ---- main loop
                def emit_av1(pc, first, last):
                    nc.tensor.matmul(
                        av_a[0:33, :],
                        lhsT=vta1_sb[:, ds(pc * 33, 33)],
                        rhs=e1_sb[:, ds((pc % ESLOTS) * 512, 512)],
                        start=first, stop=last,
                        skip_group_check=True)

                def emit_av2(pdc, first, last):
                    sl0 = (2 * pdc) % ESLOTS
                    rhs2 = e2_sb[:, ds(sl0 * 512, 1024)] \
                        .rearrange("p (s x) -> p s x", s=2)
                    nc.tensor.matmul(av_b[:, :],
                                     lhsT=vta8_v[:, pdc, :, :],
                                     rhs=rhs2,
                                     start=first, stop=last,
                                     perf_mode=DR,
                                     skip_group_check=True)

                av_a = av_b = None
                pending_av = None
                pending_drain = None

                for j in range(NJ):
                    for dc in range(ND):
                        c0, c1 = 2 * dc, 2 * dc + 1
                        for c in (c0, c1):
                            T1 = spool.tile([C, 512], F32, tag="sc",
                                            name="s1")
                            nc.tensor.matmul(T1[:, :],
                                             lhsT=xs_sb[:, ts(c, 128)],
                                             rhs=qq_v[:, j, 0, :],
                                             start=True, stop=True)
                            nc.scalar.activation(
                                e1_sb[:, ds((c % ESLOTS) * 512, 512)],
                                T1[:, :], Exp, bias=bias_sb[:, 0:1],
                                scale=ACT_SCALE)
                            T2 = spool.tile([C, 512], F32, tag="sc",
                                            name="s2")
                            nc.tensor.matmul(T2[:, :],
                                             lhsT=xs_sb[:, ts(c, 128)],
                                             rhs=qq_v[:, j, 1, :],
                                             start=True, stop=True)
                            nc.vector.tensor_scalar(
                                e2_sb[:, ds((c % ESLOTS) * 512, 512)]
                                .bitcast(U8), T2[:, :], SCH8_B, CLAMP8,
                                mybir.AluOpType.add, mybir.AluOpType.min)

                        if j == 0 and dc == 0:
                            for slab in range(8):
                                project_v(slab)

                        # avs of the previous double-chunk
                        if pending_av is not None:
                            pj, pdc = pending_av
                            if pdc == 0:
                                av_a = apool.tile([C, 512], F32, tag="a")
                                av_b = bpool.tile([C, 512], F32, tag="b")
                            first, last = (pdc == 0), (pdc == ND - 1)
                            emit_av1(2 * pdc, first=first, last=False)
                            emit_av1(2 * pdc + 1, first=False, last=last)
                            emit_av2(pdc, first=first, last=last)
                        pending_av = (j, dc)


                        if dc == 0 and pending_drain is not None:
                            pj = pending_drain
                            nc.scalar.copy(avs_sb[0:33, ts(pj, 512)],
                                           av_a[0:33, :])
                            nc.vector.tensor_copy(
                                avs_sb[64:97, ts(pj, 512)], av_b[0:33, :])
                            pending_drain = None
                        if dc == 1 and j > 0:
                            finalize_j(j - 1)
                        if dc == 10 and j + 1 < NJ:
                            project_q(j + 1, True)

                    pending_drain = j

                # flush last avs + drain
                pj, pdc = pending_av
                emit_av1(2 * pdc, first=False, last=False)
                emit_av1(2 * pdc + 1, first=False, last=True)
                emit_av2(pdc, first=False, last=True)
                nc.scalar.copy(avs_sb[0:33, ts(pj, 512)], av_a[0:33, :])
                nc.vector.tensor_copy(avs_sb[64:97, ts(pj, 512)],
                                      av_b[0:33, :])

                finalize_j(NJ - 1)

    nc.compile()
    return nc


def make_identity_input():
    ident = np.zeros((C, 33), np.float32)
    ident[0:33, :] = np.eye(33, dtype=np.float32)
    ident[64:97, :] = np.eye(33, dtype=np.float32)
    return ident


def make_in_maps(x, w_qkv):
    """Host-side sharding: subsample, fold score weights, pack per core."""
    xs = np.ascontiguousarray(x[0][:, ::R, ::R, ::R]).reshape(C, N)
    xs_b = xs.astype(NP_BF16)
    ident = make_identity_input()
    in_maps = []
    for core in range(N_CORES):
        h, half = divmod(core, 2)
        blk = w_qkv[h * 96: (h + 1) * 96].astype(np.float64)
        wq, wk, wv = blk[0:32], blk[32:64], blk[64:96]
        # folded, pre-scaled score weights: t_h = (c8 wq_h^T wk_h)^T? see
        # build_nc: lhsT for the t-projection must be W_h^T = wq_h^T wk_h
        w1t = C8 * (wq[0:DH].T @ wk[0:DH])        # [128, 128]
        w2t = C8 * (wq[DH:HD].T @ wk[DH:HD])
        w = np.empty((C, 288), np.float32)
        w[:, 0:128] = w1t
        w[:, 128:256] = w2t
        w[:, 256:288] = wv.T
        in_maps.append({
            "xs": xs_b,
            "xq": np.ascontiguousarray(xs_b[:, half * NQ:(half + 1) * NQ]),
            "w": w.astype(NP_BF16),
            "ident": ident,
        })
    return in_maps


_NC_CACHE = {}


def get_nc():
    if "nc" not in _NC_CACHE:
        _NC_CACHE["nc"] = build_nc()
    return _NC_CACHE["nc"]


LAST_RESULTS = None  # BassKernelResults of the most recent kernel() call


def kernel(x, w_qkv, trace=False, **trace_kwargs):
    global LAST_RESULTS
    x = np.asarray(x)
    w_qkv = np.asarray(w_qkv)
    in_maps = make_in_maps(x, w_qkv)
    nc = get_nc()
    res = run_bass_kernel_spmd(nc, in_maps, list(range(N_CORES)),
                               trace=trace, **trace_kwargs)
    LAST_RESULTS = res
    out_hnd = np.empty((HEADS, N, HD), np.float32)
    for core in range(N_CORES):
        h, half = divmod(core, 2)
        # device wrote [p, c, d]; row n = c*128 + p
        arr = res.results[core]["out"].reshape(C, NQ // C, HD)
        out_hnd[h, half * NQ:(half + 1) * NQ, :] = \
            arr.transpose(1, 0, 2).reshape(NQ, HD)
    return out_hnd.reshape(1, C, 16, 16, 16)


# revision 31
# speedup vs baseline: 1.0059x; 1.0055x over previous
"""DiffAttention kernel for 8 TRN2 NeuronCores (Bass/Tile).

Reference: x [1,128,32,32,32] stride-2 subsampled to xs [128, N=4096
tokens]; per head (4 heads, head_dim 32 split 16+16): diff_attn =
softmax(q1k1*scale) - 0.1*softmax(q2k2*scale); out = diff_attn @ v,
reshaped to [1,128,16,16,16].

Sharding: tensor-parallel over (head, query-half) = 8 cores. Each core
holds all 4096 tokens and computes attention for its 2048 queries.

Per-core dataflow (v2 — PE-paced, fp8 AV):
  - score weights folded on host: W1 = c8*wk1^T wq1, W2 = c8*wk2^T wq2
    (c8 = 8*log2e*scale pre-scales scores so the fp8 Schraudolph is a
    plain add).  s_h = xs_chunk^T @ (W_h @ xs_qslice): no k projection,
    one shared lhsT (the xs chunk) for both halves' score matmuls.
  - exp is whole-chunk alternated between ACT (exact exp -> fp8e4m3,
    scale ln2/8) and DVE (Schraudolph: uint8 bits = c8*s + 8*(7-c),
    round-to-nearest, min-clamp 119 to avoid inf/NaN), weighted so
    both engines run just under the PE's pace.
  - AV in fp8: av1 = vT^T @ e1 per chunk (plain fp8 matmul), av2 via
    DoubleRow over chunk PAIRS (lhsT [128,2,128] zero-padded cols
    33:128, rhs = two adjacent e-slots viewed [128,2,512]) at 0.5
    cycles/col.  A ones-column in vT accumulates the softmax sums.
  - psum: 3 score tiles [128,1024] (s1|s2 per chunk) + av1 bank +
    av2 bank = 8 banks.  av banks are single-buffered per j-block
    (start=True resets); finalize transposes reuse them between
    j-blocks.
  - keeping the PE gaplessly busy (scores+av ~1385ns per chunk pair >
    exp ~1240ns) holds the fast p-state (0.42ns/col after 3us).
"""

import math

import numpy as np
import ml_dtypes

import concourse.bass as bass
import concourse.mybir as mybir
import concourse.tile as tile
from concourse import bacc
from concourse.bass import ts, ds
from concourse.bass_utils import run_bass_kernel_spmd

BF16 = mybir.dt.bfloat16
I16 = mybir.dt.int16
F32 = mybir.dt.float32
FP8 = mybir.dt.float8e4
U8 = mybir.dt.uint8
NP_BF16 = ml_dtypes.bfloat16
NP_FP8 = ml_dtypes.float8_e4m3

C = 128          # channels
HEADS = 4
HD = 32          # head_dim
DH = 16          # d_half
LAMBDA = 0.1
SCALE = HD ** -0.5
R = 2
N_CORES = 8
N = 4096         # tokens after subsample
NQ = N // 2      # queries per core

MC = N // 128    # 32 key chunks of 128 tokens
ND = MC // 2     # 16 chunk pairs (double-chunks)
NJ = NQ // 512   # 4 j-blocks of 512 queries
NBS = 1024       # queries per finalize block (2 j-blocks)

# fp8e4m3 (ml_dtypes.float8_e4m3, IEEE-ish: bias 7, max 240, inf at
# bits 0x78): bits(2^y) ~= 8*(y + 7 - c), c balancing the Schraudolph
# sawtooth.  Scores are pre-scaled by C8 in the W weights so ACT
# recovers exp(scale*s) with scale ln2/8 and DVE just adds SCH8_B.
C8 = 8.0 * math.log2(math.e) * SCALE
ACT_SCALE = math.log(2.0) / 8.0
# global exponent shift (softmax-invariant): keeps exp values inside
# fp8e4m3 range (max 240; scores reach exp(6.5) otherwise)
SHIFT = 2.0
SCH8_B = 8.0 * (7.0 - 0.0579297) - 8.0 * math.log2(math.e) * SHIFT
CLAMP8 = 119.49
# bf16 Schraudolph for e1 chunks: bits = 16*ps + SCH16_B (psum is c8*s)
SCH16_A = 16.0
SCH16_B = 128.0 * (127.0 - 0.0579297) - 128.0 * math.log2(math.e) * SHIFT

# exp engine split: fraction of chunks on ACT (rest on DVE-Schraudolph)
ACT_SHARE = 0.54

ESLOTS = 8       # e8 ring slots (chunks)


def build_nc():
    """SPMD Bass program for one core = (head, query-half).

    Inputs:
      xs    [128, 4096] bf16  all tokens, channel-major
      w     [128, 288]  bf16  cols 0:128 W1^T, 128:256 W2^T, 256:288 w_v^T
      ident [128, 33]   f32   identity blocks at partitions 0:33, 64:97
    Output:
      out   [2048, 32]  f32   attention output (n, d) for this core's
                              queries
    """
    Exp = mybir.ActivationFunctionType.Exp
    DR = mybir.MatmulPerfMode.DoubleRow

    nc = bacc.Bacc()
    xs_d = nc.declare_dram_parameter("xs", [C, N], BF16, isOutput=False)
    xq_d = nc.declare_dram_parameter("xq", [C, NQ], BF16, isOutput=False)
    w_d = nc.declare_dram_parameter("w", [C, 288], BF16, isOutput=False)
    id_d = nc.declare_dram_parameter("ident", [C, 33], F32, isOutput=False)
    # partition-major output [p, c, d]: row n = c*128 + p maps to
    # out[p, c*HD : c*HD+HD]; host transposes back (free)
    out_d = nc.declare_dram_parameter("out", [C, (NQ // C) * HD], F32,
                                      isOutput=True)

    W1 = slice(0, 128)
    W2 = slice(128, 256)
    WV = slice(256, 288)

    with tile.TileContext(nc) as tc:
        with tc.tile_pool(name="mains", bufs=1) as mains:
            w_sb = mains.tile([C, 288], BF16)
            id_sb = mains.tile([C, 33], F32)
            xs_sb = mains.tile([C, N], BF16)
            xq_sb = mains.tile([C, NQ], BF16)

            # static tensors
            qq_sb = mains.tile([C, NJ * 2 * 512], BF16)   # t1|t2 per j
            vta1_sb = mains.tile([C, MC * 33], BF16)      # av1 weights v|1
            vta8_sb = mains.tile([C, ND * 2 * 128], FP8)  # av2 DR weights
            e1_sb = mains.tile([C, ESLOTS * 512], BF16)   # e1 ring
            e2_sb = mains.tile([C, ESLOTS * 512], FP8)    # e2 ring
            avs_sb = mains.tile([C, NJ * 512], F32)       # av1 p0:33, av2 p64:97
            out_sb = mains.tile([C, (NQ // 128) * HD], F32)

            vta8_v = vta8_sb[:, :].rearrange("p (d s m) -> p d s m",
                                             d=ND, s=2)
            qq_v = qq_sb[:, :].rearrange("p (j s n) -> p j s n",
                                         j=NJ, s=2)

            # activation bias AP (-SHIFT) for the exact-exp path
            bias_sb = mains.tile([C, 1], F32)
            nc.vector.memset(bias_sb[:, :], -SHIFT)
            # ones columns + zero DR pad cols, split across engines
            nc.vector.memset(vta1_sb[:, :], 1.0)
            nc.vector.memset(vta8_sb[:, 0:2048], 0.0)
            nc.gpsimd.memset(vta8_sb[:, 2048:4096], 0.0)
            nc.gpsimd.memset(vta8_v[:, :, :, 32:33], 1.0)

            # input DMAs spread across queues; w + first slabs first,
            # ident (finalize-only) last
            nc.sync.dma_start(out=w_sb[:, :], in_=w_d[:, :])
            nc.sync.dma_start(out=xq_sb[:, 0:512], in_=xq_d[:, 0:512])
            qengs = [nc.sync, nc.gpsimd, nc.scalar]
            for i in range(8):
                qengs[i % 3].dma_start(out=xs_sb[:, ts(i, 512)],
                                       in_=xs_d[:, ts(i, 512)])
            for i, off in enumerate((512, 1024, 1536)):
                qengs[i % 3].dma_start(out=xq_sb[:, ds(off, 512)],
                                       in_=xq_d[:, ds(off, 512)])
            nc.sync.dma_start(out=id_sb[:, :], in_=id_d[:, :])

            with (
                tc.tile_pool(name="sc_ps", bufs=6, space="PSUM") as spool,
                tc.tile_pool(name="a_ps", bufs=1, space="PSUM") as apool,
                tc.tile_pool(name="b_ps", bufs=1, space="PSUM") as bpool,
                tc.tile_pool(name="fin_sb", bufs=2) as fsb,
            ):

                def project_v_pair(pair):
                    # v^T for one chunk pair (256 tokens) -> vta (fp8)
                    pv = spool.tile([C, 512], F32, tag="sc", name="psv")
                    for i in range(2):
                        c = pair * 2 + i
                        nc.tensor.matmul(pv[:, ds(i * 128, HD)],
                                         lhsT=xs_sb[:, ts(c, 128)],
                                         rhs=w_sb[:, WV],
                                         start=True, stop=True)
                    srcv = pv[:, 0:256].rearrange(
                        "p (i x) -> p i x", x=128)[:, :, 0:HD]
                    dst1 = vta1_sb[:, ds(pair * 66, 66)].rearrange(
                        "p (c m) -> p c m", m=33)[:, :, 0:HD]
                    nc.scalar.copy(dst1, srcv)
                    dst8 = vta8_sb[:, ds(pair * 256, 256)].rearrange(
                        "p (c m) -> p c m", m=128)[:, :, 0:HD]
                    nc.scalar.copy(dst8, srcv)

                def project_v(slab):
                    # v^T for 4 chunks (512 tokens) -> vta (fp8)
                    pa = spool.tile([C, 512], F32, tag="sc", name="psv")
                    pb = spool.tile([C, 512], F32, tag="sc", name="psv")
                    ps_v = (pa, pb)
                    for i in range(4):
                        c = slab * 4 + i
                        nc.tensor.matmul(ps_v[i // 2][:, ds((i % 2) * 128,
                                                            HD)],
                                         lhsT=xs_sb[:, ts(c, 128)],
                                         rhs=w_sb[:, WV],
                                         start=True, stop=True)
                    for half in range(2):
                        srcv = ps_v[half][:, 0:256].rearrange(
                            "p (i x) -> p i x", x=128)[:, :, 0:HD]
                        coff = (slab * 4 + half * 2) * 33
                        dst1 = vta1_sb[:, ds(coff, 2 * 33)].rearrange(
                            "p (c m) -> p c m", m=33)[:, :, 0:HD]
                        nc.vector.tensor_copy(dst1, srcv)
                        dst8 = vta8_sb[:, ds(slab * 512 + half * 256, 256)] \
                            .rearrange("p (c m) -> p c m", m=128)[:, :, 0:HD]
                        nc.scalar.copy(dst8, srcv)

                def project_q(j, on_act):
                    # t1|t2 for j-block j -> qq (bf16)
                    pa = spool.tile([C, 512], F32, tag="sc", name="psq")
                    pb = spool.tile([C, 512], F32, tag="sc", name="psq")
                    qoff = j * 512
                    nc.tensor.matmul(pa[:, :], lhsT=w_sb[:, W1],
                                     rhs=xq_sb[:, ds(qoff, 512)],
                                     start=True, stop=True)
                    nc.tensor.matmul(pb[:, :], lhsT=w_sb[:, W2],
                                     rhs=xq_sb[:, ds(qoff, 512)],
                                     start=True, stop=True)
                    if on_act:
                        nc.scalar.copy(qq_sb[:, ds(j * 1024, 512)], pa[:, :])
                        nc.scalar.copy(qq_sb[:, ds(j * 1024 + 512, 512)],
                                       pb[:, :])
                    else:
                        nc.vector.tensor_copy(qq_sb[:, ds(j * 1024, 512)],
                                              pa[:, :])
                        nc.vector.tensor_copy(
                            qq_sb[:, ds(j * 1024 + 512, 512)], pb[:, :])

                def finalize_j(fj):
                    # avs_sb [33|33, fj*512 : +512] -> out rows
                    CQ = 4  # query chunks of 128 per j-block
                    pt1 = spool.tile([C, 512], F32, tag="sc", name="psT")
                    pt2 = spool.tile([C, 512], F32, tag="sc", name="psT")
                    psT1 = pt1[:, 0:256]
                    psT2 = pt2[:, 0:256]
                    for cq in range(CQ):
                        gq = fj * CQ + cq
                        nc.tensor.transpose(psT1[:, ds(cq * 64, 33)],
                                            avs_sb[0:33, ts(gq, 128)],
                                            id_sb[0:33, :])
                        nc.tensor.transpose(psT2[:, ds(cq * 64, 33)],
                                            avs_sb[64:97, ts(gq, 128)],
                                            id_sb[64:97, :])
                    r1_sb = fsb.tile([C, CQ], F32, tag="r1")
                    r2_sb = fsb.tile([C, CQ], F32, tag="r2")
                    sum1 = psT1.rearrange(
                        "p (c x) -> p c x", x=64)[:, :, 32:33]
                    sum2 = psT2.rearrange(
                        "p (c x) -> p c x", x=64)[:, :, 32:33]
                    nc.vector.reciprocal(r1_sb[:, :, None], sum1)
                    nc.vector.reciprocal(r2_sb[:, :, None], sum2)
                    nc.vector.tensor_scalar_mul(r2_sb[:, :], r2_sb[:, :],
                                                -LAMBDA)
                    o1_sb = fsb.tile([C, CQ * HD], F32, tag="o1")
                    o2_sb = fsb.tile([C, CQ * HD], F32, tag="o2")
                    av1t = psT1.rearrange(
                        "p (c x) -> p c x", x=64)[:, :, 0:32]
                    av2t = psT2.rearrange(
                        "p (c x) -> p c x", x=64)[:, :, 0:32]
                    o1_v = o1_sb[:, :].rearrange("p (c d) -> p c d", d=HD)
                    o2_v = o2_sb[:, :].rearrange("p (c d) -> p c d", d=HD)
                    nc.vector.tensor_tensor(
                        o1_v, av1t,
                        r1_sb[:, :, None].to_broadcast((C, CQ, HD)),
                        mybir.AluOpType.mult)
                    nc.vector.tensor_tensor(
                        o2_v, av2t,
                        r2_sb[:, :, None].to_broadcast((C, CQ, HD)),
                        mybir.AluOpType.mult)
                    nc.vector.tensor_tensor(
                        out_sb[:, ds(fj * CQ * HD, CQ * HD)],
                        o1_sb[:, :], o2_sb[:, :], mybir.AluOpType.add)
                    nc.sync.dma_start(
                        out=out_d[:, ds(fj * CQ * HD, CQ * HD)],
                        in_=out_sb[:, ds(fj * CQ * HD, CQ * HD)],
                    )

                # ---- preamble: q projection only; v-projs are emitted
                # right after j0/dc0's scores (they overlap dc0's exps)
                project_q(0, True)

                # ---- main loop
                def emit_av1(pc, first, last):
                    nc.tensor.matmul(
                        av_a[0:33, :],
                        lhsT=vta1_sb[:, ds(pc * 33, 33)],
                        rhs=e1_sb[:, ds((pc % ESLOTS) * 512, 512)],
                        start=first, stop=last,
                        skip_group_check=True)

                def emit_av2(pdc, first, last):
                    sl0 = (2 * pdc) % ESLOTS
                    rhs2 = e2_sb[:, ds(sl0 * 512, 1024)] \
                        .rearrange("p (s x) -> p s x", s=2)
                    nc.tensor.matmul(av_b[:, :],
                                     lhsT=vta8_v[:, pdc, :, :],
                                     rhs=rhs2,
                                     start=first, stop=last,
                                     perf_mode=DR,
                                     skip_group_check=True)

                av_a = av_b = None
                pending_av = None
                pending_drain = None

                for j in range(NJ):
                    for dc in range(ND):
                        c0, c1 = 2 * dc, 2 * dc + 1
                        for c in (c0, c1):
                            T1 = spool.tile([C, 512], F32, tag="sc",
                                            name="s1")
                            nc.tensor.matmul(T1[:, :],
                                             lhsT=xs_sb[:, ts(c, 128)],
                                             rhs=qq_v[:, j, 0, :],
                                             start=True, stop=True)
                            nc.scalar.activation(
                                e1_sb[:, ds((c % ESLOTS) * 512, 512)],
                                T1[:, :], Exp, bias=bias_sb[:, 0:1],
                                scale=ACT_SCALE)
                            T2 = spool.tile([C, 512], F32, tag="sc",
                                            name="s2")
                            nc.tensor.matmul(T2[:, :],
                                             lhsT=xs_sb[:, ts(c, 128)],
                                             rhs=qq_v[:, j, 1, :],
                                             start=True, stop=True)
                            nc.vector.tensor_scalar(
                                e2_sb[:, ds((c % ESLOTS) * 512, 512)]
                                .bitcast(U8), T2[:, :], SCH8_B, CLAMP8,
                                mybir.AluOpType.add, mybir.AluOpType.min)

                        if j == 0 and dc == 0:
                            for slab in range(8):
                                project_v(slab)

                        # avs of the previous double-chunk
                        if pending_av is not None:
                            pj, pdc = pending_av
                            if pdc == 0:
                                av_a = apool.tile([C, 512], F32, tag="a")
                                av_b = bpool.tile([C, 512], F32, tag="b")
                            first, last = (pdc == 0), (pdc == ND - 1)
                            emit_av1(2 * pdc, first=first, last=False)
                            emit_av1(2 * pdc + 1, first=False, last=last)
                            emit_av2(pdc, first=first, last=last)
                        pending_av = (j, dc)


                        if dc == 0 and pending_drain is not None:
                            pj = pending_drain
                            nc.scalar.copy(avs_sb[0:33, ts(pj, 512)],
                                           av_a[0:33, :])
                            nc.vector.tensor_copy(
                                avs_sb[64:97, ts(pj, 512)], av_b[0:33, :])
                            pending_drain = None
                        if dc == 1 and j > 0:
                            finalize_j(j - 1)
                        if dc == 10 and j + 1 < NJ:
                            project_q(j + 1, True)

                    pending_drain = j

                # flush last avs + drain
                pj, pdc = pending_av
                emit_av1(2 * pdc, first=False, last=False)
                emit_av1(2 * pdc + 1, first=False, last=True)
                emit_av2(pdc, first=False, last=True)
                nc.scalar.copy(avs_sb[0:33, ts(pj, 512)], av_a[0:33, :])
                nc.vector.tensor_copy(avs_sb[64:97, ts(pj, 512)],
                                      av_b[0:33, :])

                finalize_j(NJ - 1)

    nc.compile()
    return nc


def make_identity_input():
    ident = np.zeros((C, 33), np.float32)
    ident[0:33, :] = np.eye(33, dtype=np.float32)
    ident[64:97, :] = np.eye(33, dtype=np.float32)
    return ident


def make_in_maps(x, w_qkv):
    """Host-side sharding: subsample, fold score weights, pack per core."""
    xs = np.ascontiguousarray(x[0][:, ::R, ::R, ::R]).reshape(C, N)
    xs_b = xs.astype(NP_BF16)
    ident = make_identity_input()
    in_maps = []
    for core in range(N_CORES):
        h, half = divmod(core, 2)
        blk = w_qkv[h * 96: (h + 1) * 96].astype(np.float64)
        wq, wk, wv = blk[0:32], blk[32:64], blk[64:96]
        # folded, pre-scaled score weights: t_h = (c8 wq_h^T wk_h)^T? see
        # build_nc: lhsT for the t-projection must be W_h^T = wq_h^T wk_h
        w1t = C8 * (wq[0:DH].T @ wk[0:DH])        # [128, 128]
        w2t = C8 * (wq[DH:HD].T @ wk[DH:HD])
        w = np.empty((C, 288), np.float32)
        w[:, 0:128] = w1t
        w[:, 128:256] = w2t
        w[:, 256:288] = wv.T
        in_maps.append({
            "xs": xs_b,
            "xq": np.ascontiguousarray(xs_b[:, half * NQ:(half + 1) * NQ]),
            "w": w.astype(NP_BF16),
            "ident": ident,
        })
    return in_maps


_NC_CACHE = {}


def get_nc():
    if "nc" not in _NC_CACHE:
        _NC_CACHE["nc"] = build_nc()
    return _NC_CACHE["nc"]


LAST_RESULTS = None  # BassKernelResults of the most recent kernel() call


def kernel(x, w_qkv, trace=False, **trace_kwargs):
    global LAST_RESULTS
    x = np.asarray(x)
    w_qkv = np.asarray(w_qkv)
    in_maps = make_in_maps(x, w_qkv)
    nc = get_nc()
    res = run_bass_kernel_spmd(nc, in_maps, list(range(N_CORES)),
                               trace=trace, **trace_kwargs)
    LAST_RESULTS = res
    out_hnd = np.empty((HEADS, N, HD), np.float32)
    for core in range(N_CORES):
        h, half = divmod(core, 2)
        # device wrote [p, c, d]; row n = c*128 + p
        arr = res.results[core]["out"].reshape(C, NQ // C, HD)
        out_hnd[h, half * NQ:(half + 1) * NQ, :] = \
            arr.transpose(1, 0, 2).reshape(NQ, HD)
    return out_hnd.reshape(1, C, 16, 16, 16)
